# revision 26
# baseline (speedup 1.0000x reference)
"""Trainium2 Bass kernel for the Griffin-style gated linear recurrence.

Model (matching the jax reference, including its chunked-scan numerics):
    a = sigmoid(x @ Wa.T + decay_bias)
    i = sigmoid(x @ Wi.T)
    v = x @ Wv.T
    w = sqrt(max(1 - a*a, 1e-8)) * i * v
    chunked scan (chunk=64): equivalent to h[t] = a[t]*h[t-1] + g[t]*w[t]
    with g[t] = min(1, cd[t]*1e10), cd = within-chunk running product of a.

Sharding: 4 batches x 2 channel-halves = 8 cores, no communication.
Per core: x[b] as [1024, 4096] fp16, stacked weight shard [1024, 576] fp16
(cols: a0|i0|v0|[a1,i1]|v1), output [192, 4096] fp16 (host upcasts).

Blocks of 512 steps are processed in PAIRS; all SBUF-side elementwise work
runs on pair-wide [P, 1024] tiles to halve instruction overheads, and the
h recurrence scan chains naturally across the pair.

Engine plan per pair:
  PE    : 2 x 5 channel groups x 8 k-tiles (fp16, 1 cyc/row).  PSUM banks:
          sigmoid-fed groups rotate over 4 banks, v groups over the other 4
          (v is consumed latest), so the PE never waits on a bank.
  ACT   : 3 sigmoids per block ([a1|i1] share one bank+instr), pair-wide
          squares (in every act table) and sqrts.  A 1-element Copy reading
          the pair's last sigmoid output produces the sqrt bias tile (==1.0),
          forcing sqrts to schedule after the sigmoids: 2 table loads/pair.
  DVE   : u = i*v (PSUM reads); the chunked gate via ONE pair-wide scan:
          with M = a*mask (mask = 1e10 at chunk starts, 0 elsewhere),
          C[t] = max(a[t]*C[t-1], M[t]) equals 1e10 * within-chunk running
          product exactly (state<=1e10 so the max is a hard reset at chunk
          starts); then gw = min(C,1)*w in one fused scalar_tensor_tensor;
          finally the pair-wide h recurrence scan.
  Pool  : M = a*mask and w = r*u products (SBUF-only engine).
"""

import sys

if "/opt/trn_rl_repo" not in sys.path:
    sys.path.insert(0, "/opt/trn_rl_repo")

from contextlib import ExitStack

import numpy as np

from concourse import bacc, bass, mybir, tile
from concourse.bass_utils import run_bass_kernel_spmd

B, S = 4, 4096
DM, DR = 1024, 384
DC = DR // 2          # channels per core
CH = 64               # scan chunk size
SB = 512              # sequence block (one PSUM tile)
PB = 2 * SB           # pair block for SBUF-side work
NB = S // SB
KT = DM // 128        # contraction tiles
WC = 576              # stacked weight columns: a0|i0|v0|[a1,i1]|v1

F32 = mybir.dt.float32
F16 = mybir.dt.float16
AFT = mybir.ActivationFunctionType
OP = mybir.AluOpType

# column ranges of the stacked weight / PSUM group layout
GCOLS = ((0, 128), (128, 256), (256, 384), (384, 512), (512, 576))

_CACHED_NC = None


def _build_nc():
    nc = bacc.Bacc(trn_type="TRN2")

    xT = nc.dram_tensor("xt", [DM, S], F16, kind="ExternalInput")
    wT = nc.dram_tensor("wt", [DM, WC], F16, kind="ExternalInput")
    bias = nc.dram_tensor("biasa", [128, 2], F32, kind="ExternalInput")
    out = nc.dram_tensor("out", [DC, S], F16, kind="ExternalOutput")

    with tile.TileContext(nc) as tc, ExitStack() as ctx:
        wp = ctx.enter_context(tc.tile_pool(name="wp", bufs=1))
        cp = ctx.enter_context(tc.tile_pool(name="cp", bufs=1))
        xp = ctx.enter_context(tc.tile_pool(name="xp", bufs=4))
        pp = ctx.enter_context(tc.tile_pool(name="pp", bufs=1, space="PSUM"))
        sp = ctx.enter_context(tc.tile_pool(name="sp", bufs=2))
        hp = ctx.enter_context(tc.tile_pool(name="hp", bufs=2))

        # --- x prefetch + constants ------------------------------------
        # first x block goes out before the weights so the PE can start
        # as soon as both arrive; remaining blocks stream behind.
        x_tiles = {}

        def fetch_x(ib):
            x_sb = xp.tile([128, KT, SB], F16, tag="x", name=f"x{ib}")
            nc.sync.dma_start(
                x_sb[:],
                xT.rearrange("(k p) s -> p k s", p=128)
                [:, :, ib * SB:(ib + 1) * SB])
            x_tiles[ib] = x_sb

        # group-0 weights and the first x block first (all the first matmul
        # needs), then the rest; later x blocks prefetch from the pipeline
        w_sb = wp.tile([128, KT, WC], F16, tag="w")
        wr = wT.rearrange("(k p) c -> p k c", p=128)
        fetch_x(0)
        nc.sync.dma_start(w_sb[:, :, 0:128], wr[:, :, 0:128])
        nc.sync.dma_start(w_sb[:, :, 128:WC], wr[:, :, 128:WC])
        fetch_x(1)

        # dummy matmuls on scratch data: keep the PE busy during the input
        # DMA so the HAM clock-gate is fully ramped when real work arrives
        scr = wp.tile([128, SB], F16, tag="scr")
        nc.vector.memset(scr[:], 0.0)
        zd = pp.tile([128, SB], F32, tag="s0")
        for _ in range(32):
            nc.tensor.matmul(zd[:], scr[:, 0:128], scr[:], start=True,
                             stop=True)

        # bias columns: col 0 = decay_bias[0:128]; col 1 = [bias[128:192]; 0]
        bt = cp.tile([128, 2], F32, tag="bt")
        nc.sync.dma_start(bt[:], bias[:, :])



        def front_half(ib, half, a0p, ai1p, u0p, u1p, sub0=0, subw=SB,
                       rot=None):
            """Matmuls + sigmoids + u for columns [sub0, sub0+subw) of
            block ib, landing at pair-tile columns half*SB + sub0."""
            if ib + 2 < NB and ib + 2 not in x_tiles:
                fetch_x(ib + 2)
            cs = slice(half * SB + sub0, half * SB + sub0 + subw)
            x_sb = x_tiles[ib]
            if rot is None:
                rot = ib

            # PSUM: sigmoid-fed groups (j 0,1,3) rotate over banks s0..s3,
            # v groups (j 2,4) over banks v0..v3 (freed latest by DVE u).
            zp = []
            for j, (c0, c1) in enumerate(GCOLS):
                if j in (0, 1, 3):
                    tag = f"s{(rot * 3 + (0, 1, None, 2, None)[j]) % 4}"
                else:
                    tag = f"v{(rot * 2 + (None, None, 0, None, 1)[j]) % 4}"
                zt = pp.tile([128, SB], F32, tag=tag)
                z = zt[0:c1 - c0, 0:subw]
                for k in range(KT):
                    nc.tensor.matmul(
                        z,
                        w_sb[:, k, c0:c1],
                        x_sb[:, k, sub0:sub0 + subw],
                        start=(k == 0),
                        stop=(k == KT - 1),
                    )
                zp.append(z)
            za0, zi0, zv0, zai1, zv1 = zp

            i0 = sp.tile([128, PB], F16, tag="i0")
            nc.scalar.activation(a0p[:, cs], za0, AFT.Sigmoid, bias=bt[:, 0:1])
            nc.scalar.activation(i0[:, cs], zi0, AFT.Sigmoid)
            # one sigmoid for the [a1|i1] bank; bias col1 = [b1;0]
            nc.scalar.activation(ai1p[:, cs], zai1, AFT.Sigmoid, bias=bt[:, 1:2])

            # v0 out of PSUM via ACT Copy (in every act table set, so no
            # table reload), then u0 = i0*v0 on the Pool engine.  u1 stays
            # on DVE: its i-operand sits at partition base 64, legal only
            # against a PSUM operand.
            v0c = sp.tile([128, SB], F16, tag="v0c")
            nc.scalar.activation(v0c[:, 0:subw], zv0, AFT.Copy)
            nc.gpsimd.tensor_tensor(
                u0p[:, cs], i0[:, cs], v0c[:, 0:subw], OP.mult)
            nc.vector.tensor_tensor(
                u1p[:, cs], ai1p[64:128, cs], zv1[0:64, :], OP.mult)
            return ai1p

        prev_h = None
        # blocks mostly in pairs; the last two run alone so the end-of-kernel
        # drain chain works on smaller tiles
        plan = [(0, 1), (2, 3), (4, 5), (6,), (7,)]
        for p, entry in enumerate(plan):
            nb = len(entry)
            used = nb * SB
            # pair-wide fp16 tiles: [:, 0:512] = block A, [:, 512:1024] = B
            a0p = sp.tile([128, PB], F16, tag="a0p")
            ai1p = sp.tile([128, PB], F16, tag="ai1p")
            u0p = sp.tile([128, PB], F16, tag="u0p")
            u1p = sp.tile([64, PB], F16, tag="u1p")

            last_entry = p == len(plan) - 1
            for half, ib in enumerate(entry):
                if last_entry:
                    # final block in two 256-wide sub-fronts: the drain
                    # chain after the very last matmul stays short
                    front_half(ib, half, a0p, ai1p, u0p, u1p, 0, 256, ib)
                    front_half(ib, half, a0p, ai1p, u0p, u1p, 256, 256, ib + 1)
                else:
                    front_half(ib, half, a0p, ai1p, u0p, u1p)

            # 1-element Copy reading the entry's last sigmoid output;
            # produces the all-ones sqrt bias column and pins sqrts after
            # sigmoids (2 act table loads per entry).
            gate = sp.tile([128, 1], F32, tag="gate")
            nc.scalar.activation(
                gate[:], ai1p[:, used - 1:used], AFT.Copy, bias=1.0, scale=0.0)

            a1p = ai1p[0:64, :]
            if last_entry:
                col_slices = [slice(0, 256), slice(256, 512)]
            elif p == len(plan) - 2:
                col_slices = [slice(0, used)]
            else:
                col_slices = [slice(0, used)]
            s_base = entry[0] * SB
            new_h = {}
            for gi, (ap, up, P) in enumerate(
                    ((a0p, u0p, 128), (a1p, u1p, 64))):
                # m = a*a on ACT: Square is in every act table set
                m = sp.tile([P, PB], F16, tag=f"m{gi}")
                r = sp.tile([P, PB], F16, tag=f"r{gi}")
                w = sp.tile([P, PB], F16, tag=f"w{gi}")
                mm = sp.tile([P, PB], F32, tag=f"mm{gi}")
                cc = sp.tile([P, PB], F32, tag=f"cc{gi}")
                gw = sp.tile([P, PB], F16, tag=f"gw{gi}")
                h = hp.tile([P, PB], F16, tag=f"h{gi}")
                # M (= a*1e10 at chunk starts, 0 elsewhere) only needs its
                # reset columns written: the mm buffers are zeroed on first
                # use (first two plan entries seed both rotating buffers),
                # then only the strided columns are updated.
                if p < 2:
                    nc.vector.memset(mm[:], 0.0)
                init_col = None if prev_h is None else prev_h[gi]
                for cs in col_slices:
                    nc.scalar.activation(m[:, cs], ap[:, cs], AFT.Square)
                    # r = sqrt(gate*1 - m) = sqrt(1 - a*a)
                    nc.scalar.activation(
                        r[:, cs], m[:, cs], AFT.Sqrt,
                        bias=gate[0:P, :], scale=-1.0)
                    # w = r*u: Pool in steady state, DVE (faster per-op) on
                    # the drain-critical final entries
                    if last_entry:
                        nc.vector.tensor_tensor(
                            w[:, cs], r[:, cs], up[:, cs], OP.mult)
                    else:
                        nc.gpsimd.tensor_tensor(
                            w[:, cs], r[:, cs], up[:, cs], OP.mult)
                    # C = 1e10 * within-chunk running product of a (one scan)
                    av = ap[:, cs].rearrange("q (c u) -> q c u", u=CH)[:, :, 0]
                    mv = mm[:, cs].rearrange("q (c u) -> q c u", u=CH)[:, :, 0]
                    nc.vector.tensor_scalar(mv, av, 1e10, None, op0=OP.mult)
                    nc.vector.tensor_tensor_scan(
                        cc[:, cs], ap[:, cs], mm[:, cs], 0.0,
                        op0=OP.mult, op1=OP.max
                    )
                    # gw = min(C, 1) * w  (fused)
                    nc.vector.scalar_tensor_tensor(
                        gw[:, cs], cc[:, cs], 1.0, w[:, cs],
                        op0=OP.min, op1=OP.mult
                    )
                    init = 0.0 if init_col is None else init_col
                    nc.vector.tensor_tensor_scan(
                        h[:, cs], ap[:, cs], gw[:, cs], init,
                        op0=OP.mult, op1=OP.add
                    )
                    init_col = h[:, cs.stop - 1:cs.stop]
                    c0 = 0 if gi == 0 else 128
                    nc.sync.dma_start(
                        out[c0:c0 + P,
                            s_base + cs.start:s_base + cs.stop],
                        h[:, cs])
                new_h[gi] = init_col
            prev_h = new_h

    nc.finalize()
    return nc


def _make_in_maps(x, Wa, Wi, Wv, decay_bias):
    x = np.asarray(x, dtype=np.float32)
    Wa = np.asarray(Wa, dtype=np.float32)
    Wi = np.asarray(Wi, dtype=np.float32)
    Wv = np.asarray(Wv, dtype=np.float32)
    decay_bias = np.asarray(decay_bias, dtype=np.float32)

    in_maps = []
    for b in range(B):
        xTb = np.ascontiguousarray(x[b].T.astype(np.float16))   # [DM, S]
        for j in range(2):
            c0 = j * DC
            # stacked weight [DM, 576]: a0 | i0 | v0 | a1,i1 | v1
            wcat = np.concatenate([
                Wa[c0:c0 + 128].T,
                Wi[c0:c0 + 128].T,
                Wv[c0:c0 + 128].T,
                Wa[c0 + 128:c0 + DC].T,
                Wi[c0 + 128:c0 + DC].T,
                Wv[c0 + 128:c0 + DC].T,
            ], axis=1).astype(np.float16)
            bcols = np.zeros((128, 2), dtype=np.float32)
            bcols[:, 0] = decay_bias[c0:c0 + 128]
            bcols[0:64, 1] = decay_bias[c0 + 128:c0 + DC]
            in_maps.append({
                "xt": xTb,
                "wt": np.ascontiguousarray(wcat),
                "biasa": bcols,
            })
    return in_maps


def kernel(x, Wa, Wi, Wv, decay_bias):
    global _CACHED_NC
    if _CACHED_NC is None:
        _CACHED_NC = _build_nc()
    nc = _CACHED_NC

    in_maps = _make_in_maps(x, Wa, Wi, Wv, decay_bias)
    res = run_bass_kernel_spmd(nc, in_maps, core_ids=list(range(8)))

    out = np.empty((B, S, DR), dtype=np.float32)
    for b in range(B):
        for j in range(2):
            core = 2 * b + j
            out[b, :, j * DC:(j + 1) * DC] = \
                res.results[core]["out"].T.astype(np.float32)
    return out
